# revision 7
# baseline (speedup 1.0000x reference)
"""Trainium2 Bass kernel for GCNCriticNet (gnn_message_passing).

Graphs are 8192 independent complete graphs of 16 nodes (+ self loops), so
every node has degree 16, the symmetric norm is 1/16, and GCN aggregation
collapses to a per-graph mean. Edge lists never reach the device.

Math per core (16384 nodes = 1024 graphs, feature-major [128, nodes]):
  u1 = Wcomb^T Z          Z = [obs ; bcast(graph-sum obs)]  (K=128 stacked)
  x1 = tanh(u1 + b1f)                                       (ACT, exact)
  h2 = W2s^T sum_s x1     (16 accumulating matmuls, same weights)
  u2 = x1 + bcast(h2 + b2)                                  (GPSIMD STT)
  x2 = tanh(u2)                                             (ACT, exact)
  y  = wfc^T sum_s x2     (16 accumulating matmuls)
Host: out = y + b_fc1 (the /16 of the mean is folded into wfc).

All matmuls bf16 (1 cyc/row vs 4 for fp32). obs is pre-transposed on host to
feature-major bf16 so no on-chip transposes are needed and DMA halves.
Group sums of obs: DVE pairwise tree (bf16 2x mode) + log-doubling broadcast
into the Z tile bottom. Per chunk of 1024 nodes, software-pipelined.
"""

import sys
import numpy as np

try:
    import concourse.bass as bass  # noqa: F401
except ImportError:  # harness runs in a bare dir; repo is on the box
    for p in ("/opt/trn_rl_repo", "/root/.axon_site/_ro/trn_rl_repo"):
        if p not in sys.path:
            sys.path.insert(0, p)
    import concourse.bass as bass  # noqa: F401

import ml_dtypes
import concourse.bacc as bacc
import concourse.mybir as mybir
import concourse.tile as tile
from concourse.bass import MemorySpace
from concourse.bass_utils import run_bass_kernel_spmd

F32 = mybir.dt.float32
BF16 = mybir.dt.bfloat16
AF = mybir.ActivationFunctionType
ALU = mybir.AluOpType

N_CORES = 8
N_AGENTS = 16
BATCH = 8192
OBS = 64
HID = 128
N = BATCH * N_AGENTS            # 131072 nodes
NPC = N // N_CORES              # 16384 nodes / core
CHUNK = 1024                    # nodes per inner iteration
NCHUNK = NPC // CHUNK           # 16
GPC = CHUNK // N_AGENTS         # 64 graphs per chunk
OUTPC = NPC // N_AGENTS         # 1024 graphs per core

_CACHE = {}


def _build_nc():
    nc = bacc.Bacc("TRN2", target_bir_lowering=False, debug=False)

    obs_d = nc.dram_tensor("obs", [NCHUNK, OBS, CHUNK], BF16, kind="ExternalInput")
    wcomb_d = nc.dram_tensor("wcomb", [128, HID], BF16, kind="ExternalInput")
    w2_d = nc.dram_tensor("w2s", [HID, HID], BF16, kind="ExternalInput")
    wfc_d = nc.dram_tensor("wfc", [HID, 1], BF16, kind="ExternalInput")
    b1f_d = nc.dram_tensor("b1f", [HID, 1], F32, kind="ExternalInput")
    b2_d = nc.dram_tensor("b2", [HID, 1], F32, kind="ExternalInput")
    out_d = nc.dram_tensor("out", [1, OUTPC], F32, kind="ExternalOutput")

    S = N_AGENTS

    with tile.TileContext(nc) as tc:
        with (
            tc.tile_pool(name="const", bufs=1) as cp,
            tc.tile_pool(name="zt", bufs=3) as ztp,
            tc.tile_pool(name="sc", bufs=2) as scp,
            tc.tile_pool(name="x1p", bufs=3) as x1p,
            tc.tile_pool(name="u2p", bufs=2) as u2p,
            tc.tile_pool(name="x2p", bufs=3) as x2p,
            tc.tile_pool(name="hsp", bufs=2) as hsp,
            tc.tile_pool(name="pu1", bufs=2, space=MemorySpace.PSUM) as pu1,
            tc.tile_pool(name="psm", bufs=2, space=MemorySpace.PSUM) as psm,
        ):
            wcomb = cp.tile([128, HID], BF16)
            nc.sync.dma_start(wcomb[:], wcomb_d[:])
            w2 = cp.tile([HID, HID], BF16)
            nc.sync.dma_start(w2[:], w2_d[:])
            wfc = cp.tile([HID, 1], BF16)
            nc.sync.dma_start(wfc[:], wfc_d[:])
            b1f = cp.tile([HID, 1], F32)
            nc.sync.dma_start(b1f[:], b1f_d[:])
            b2 = cp.tile([HID, 1], F32)
            nc.sync.dma_start(b2[:], b2_d[:])
            outsb = cp.tile([1, OUTPC], F32)

            zt_of, u1_of, x1_of, hs_of, u2_of, x2_of, yp_of = (
                {}, {}, {}, {}, {}, {}, {}
            )

            def stage_a(c):
                # DMA obs chunk into Z top; DVE tree -> graph sums -> Z bottom
                zt = ztp.tile([128, CHUNK], BF16, tag="zt")
                zt_of[c] = zt
                for q in range(4):
                    nc.sync.dma_start(
                        zt[q * 16:(q + 1) * 16, :], obs_d[c, q * 16:(q + 1) * 16, :]
                    )
                top = zt[0:64, :].rearrange("p (g s) -> p g s", s=S)
                t1 = scp.tile([64, CHUNK // 2], BF16, tag="t1")
                v1 = t1[:].rearrange("p (g s) -> p g s", s=8)
                nc.vector.tensor_add(v1, top[:, :, 0:8], top[:, :, 8:16])
                t2 = scp.tile([64, CHUNK // 4], BF16, tag="t2")
                v2 = t2[:].rearrange("p (g s) -> p g s", s=4)
                nc.vector.tensor_add(v2, v1[:, :, 0:4], v1[:, :, 4:8])
                t3 = scp.tile([64, CHUNK // 8], BF16, tag="t3")
                v3 = t3[:].rearrange("p (g s) -> p g s", s=2)
                nc.vector.tensor_add(v3, v2[:, :, 0:2], v2[:, :, 2:4])
                # final round writes straight into Z bottom slot s=0
                bot = zt[64:128, :].rearrange("p (g s) -> p g s", s=S)
                nc.vector.tensor_add(bot[:, :, 0:1], v3[:, :, 0:1], v3[:, :, 1:2])
                # log-double broadcast within each 16-slot group
                nc.vector.tensor_copy(bot[:, :, 1:2], bot[:, :, 0:1])
                nc.vector.tensor_copy(bot[:, :, 2:4], bot[:, :, 0:2])
                nc.vector.tensor_copy(bot[:, :, 4:8], bot[:, :, 0:4])
                nc.vector.tensor_copy(bot[:, :, 8:16], bot[:, :, 0:8])

            def stage_b(c):
                zt = zt_of.pop(c)
                u1 = pu1.tile([HID, CHUNK], F32, tag="u1")
                u1_of[c] = u1
                nc.tensor.matmul(u1[:, 0:512], wcomb[:], zt[:, 0:512],
                                 start=True, stop=True)
                nc.tensor.matmul(u1[:, 512:1024], wcomb[:], zt[:, 512:1024],
                                 start=True, stop=True)

            def stage_c(c):
                u1 = u1_of.pop(c)
                x1 = x1p.tile([HID, CHUNK], BF16, tag="x1")
                x1_of[c] = x1
                nc.scalar.activation(x1[:], u1[:], AF.Tanh, bias=b1f[:])

            def stage_d(c):
                # h2 = W2s^T sum_s x1 : 16 accumulating matmuls, same weights
                x1 = x1_of[c]
                xv = x1[:].rearrange("p (g s) -> p g s", s=S)
                h2p = psm.tile([HID, GPC], F32, tag="h2")
                for s in range(S):
                    nc.tensor.matmul(h2p[:], w2[:], xv[:, :, s],
                                     start=(s == 0), stop=(s == S - 1),
                                     skip_group_check=(s > 0))
                h2s = hsp.tile([HID, GPC], BF16, tag="h2s")
                hs_of[c] = h2s
                nc.vector.tensor_scalar_add(h2s[:], h2p[:], b2[:])

            def stage_e(c):
                x1 = x1_of.pop(c)
                h2s = hs_of.pop(c)
                u2 = u2p.tile([HID, CHUNK], BF16, tag="u2")
                u2_of[c] = u2
                hb = h2s[:].rearrange("p (g o) -> p g o", o=1).broadcast_to(
                    [HID, GPC, S]
                )
                xv = x1[:].rearrange("p (g s) -> p g s", s=S)
                uv = u2[:].rearrange("p (g s) -> p g s", s=S)
                half = GPC // 2
                nc.gpsimd.tensor_add(uv[:, 0:half], xv[:, 0:half], hb[:, 0:half])
                nc.gpsimd.tensor_add(uv[:, half:GPC], xv[:, half:GPC], hb[:, half:GPC])

            def stage_f(c):
                u2 = u2_of.pop(c)
                x2 = x2p.tile([HID, CHUNK], BF16, tag="x2")
                x2_of[c] = x2
                nc.scalar.activation(x2[:, 0:512], u2[:, 0:512], AF.Tanh)
                nc.scalar.activation(x2[:, 512:1024], u2[:, 512:1024], AF.Tanh)

            def stage_g(c):
                x2 = x2_of.pop(c)
                xv = x2[:].rearrange("p (g s) -> p g s", s=S)
                yp = psm.tile([1, GPC], F32, tag="y")
                for s in range(S):
                    nc.tensor.matmul(yp[:], wfc[:], xv[:, :, s],
                                     start=(s == 0), stop=(s == S - 1),
                                     skip_group_check=(s > 0))
                nc.vector.tensor_copy(outsb[0:1, c * GPC:(c + 1) * GPC], yp[:])

            # software pipeline
            stage_a(0); stage_a(1)
            stage_b(0)
            stage_a(2); stage_b(1)
            stage_c(0); stage_c(1)
            for c in range(NCHUNK):
                if c + 3 < NCHUNK:
                    stage_a(c + 3)
                if c + 2 < NCHUNK:
                    stage_b(c + 2)
                    stage_c(c + 2)
                stage_d(c)
                stage_e(c)
                stage_f(c)
                stage_g(c)

            nc.sync.dma_start(out_d[:], outsb[:])

    nc.compile()
    return nc


def _get_nc():
    if "nc" not in _CACHE:
        _CACHE["nc"] = _build_nc()
    return _CACHE["nc"]


def _make_in_maps(cent_obs, w_emb, b_emb, w_gcn, b_gcn, w_fc1):
    w_emb = np.ascontiguousarray(w_emb, np.float32)
    wcomb = np.concatenate(
        [w_emb, (w_emb @ w_gcn[0]) / np.float32(16.0)], axis=0
    ).astype(ml_dtypes.bfloat16)
    w2s = (w_gcn[1] / np.float32(16.0)).astype(ml_dtypes.bfloat16)
    wfc = (w_fc1.reshape(HID, 1) / np.float32(16.0)).astype(ml_dtypes.bfloat16)
    b1f = (b_gcn[0] + b_emb + b_emb @ w_gcn[0]).astype(np.float32).reshape(HID, 1)
    b2 = b_gcn[1].astype(np.float32).reshape(HID, 1)
    shared = {"wcomb": wcomb, "w2s": w2s, "wfc": wfc, "b1f": b1f, "b2": b2}
    obs_all = np.ascontiguousarray(cent_obs, np.float32).astype(ml_dtypes.bfloat16)
    obs_all = obs_all.reshape(N_CORES, NCHUNK, CHUNK, OBS).transpose(0, 1, 3, 2)
    in_maps = []
    for ci in range(N_CORES):
        m = dict(shared)
        m["obs"] = np.ascontiguousarray(obs_all[ci])
        in_maps.append(m)
    return in_maps


def kernel(cent_obs, w_emb, b_emb, w_gcn, b_gcn, w_fc1, b_fc1,
           edge_src, edge_dst, _trace=False):
    cent_obs = np.asarray(cent_obs, np.float32)
    nc = _get_nc()
    in_maps = _make_in_maps(
        cent_obs, np.asarray(w_emb, np.float32), np.asarray(b_emb, np.float32),
        np.asarray(w_gcn, np.float32), np.asarray(b_gcn, np.float32),
        np.asarray(w_fc1, np.float32),
    )
    kw = dict(trace=True) if _trace else {}
    res = run_bass_kernel_spmd(nc, in_maps, list(range(N_CORES)), **kw)
    y = np.concatenate(
        [np.asarray(res.results[i]["out"]).reshape(-1) for i in range(N_CORES)]
    )
    out = (y + np.float32(np.asarray(b_fc1).reshape(()))).astype(np.float32)
    if _trace:
        _CACHE["last_result"] = res
    return out.reshape(BATCH, 1)


# revision 11
# speedup vs baseline: 1.2166x; 1.2166x over previous
"""Trainium2 Bass kernel for GCNCriticNet (gnn_message_passing).

Graphs are 8192 independent complete graphs of 16 nodes (+ self loops), so
every node has degree 16, the symmetric norm is 1/16, and GCN aggregation
collapses to a per-graph mean. Edge lists never reach the device.

Per core (16384 nodes = 1024 graphs), feature-major [128, node-cols], node
columns ordered (s, g) — node-within-graph major — so every reduction /
broadcast is a fully contiguous DVE op:
  u1 = Wcomb^T Z        Z = [obs ; bcast(graph-sum obs)]  (K=128 stacked)
  x1 = tanh(u1 + b1f)                                     ACT
  sx1 = group-sum(x1)   pairwise tree on DVE (bf16 2x)
  h2  = W2s^T sx1       one matmul;  h2s = h2 + b2        DVE
  u2  = x1 + bcast(h2s)                                   GPSIMD
  x2  = tanh(u2)                                          ACT
  y   = wfc^T group-sum(x2)                               DVE tree + matmul
Host: out = y + b_fc1 (mean's /16 folded into weights).

Processed in 8 macro-chunks of 2048 nodes (128 graphs); u1/tanh1/u2/tanh2
split in 1024-col halves to bound PSUM usage and pipeline ACT.
"""

import sys
import numpy as np

try:
    import concourse.bass as bass  # noqa: F401
except ImportError:  # harness runs in a bare dir; repo is on the box
    for p in ("/opt/trn_rl_repo", "/root/.axon_site/_ro/trn_rl_repo"):
        if p not in sys.path:
            sys.path.insert(0, p)
    import concourse.bass as bass  # noqa: F401

import ml_dtypes
import concourse.bacc as bacc
import concourse.mybir as mybir
import concourse.tile as tile
from concourse.bass import MemorySpace
from concourse.bass_utils import run_bass_kernel_spmd

F32 = mybir.dt.float32
BF16 = mybir.dt.bfloat16
AF = mybir.ActivationFunctionType

N_CORES = 8
N_AGENTS = 16
BATCH = 8192
OBS = 64
HID = 128
N = BATCH * N_AGENTS            # 131072 nodes
NPC = N // N_CORES              # 16384 nodes / core
MC = 2048                       # nodes per macro-chunk
NMC = NPC // MC                 # 8
GPM = MC // N_AGENTS            # 128 graphs per macro
OUTPC = NPC // N_AGENTS         # 1024 graphs per core
S = N_AGENTS

_CACHE = {}


def _build_nc():
    nc = bacc.Bacc("TRN2", target_bir_lowering=False, debug=False)

    obs_d = nc.dram_tensor("obs", [NMC, OBS, MC], BF16, kind="ExternalInput")
    wcomb_d = nc.dram_tensor("wcomb", [128, HID], BF16, kind="ExternalInput")
    w2_d = nc.dram_tensor("w2s", [HID, HID], BF16, kind="ExternalInput")
    wfc_d = nc.dram_tensor("wfc", [HID, 1], BF16, kind="ExternalInput")
    b1f_d = nc.dram_tensor("b1f", [HID, 1], F32, kind="ExternalInput")
    b2_d = nc.dram_tensor("b2", [HID, 1], F32, kind="ExternalInput")
    out_d = nc.dram_tensor("out", [1, OUTPC], F32, kind="ExternalOutput")

    with tile.TileContext(nc) as tc:
        with (
            tc.tile_pool(name="const", bufs=1) as cp,
            tc.tile_pool(name="zt", bufs=3) as ztp,
            tc.tile_pool(name="sc", bufs=2) as scp,
            tc.tile_pool(name="x1p", bufs=3) as x1p,
            tc.tile_pool(name="u2p", bufs=2) as u2p,
            tc.tile_pool(name="x2p", bufs=2) as x2p,
            tc.tile_pool(name="hsp", bufs=2) as hsp,
            tc.tile_pool(name="pu1", bufs=2, space=MemorySpace.PSUM) as pu1,
            tc.tile_pool(name="psm", bufs=2, space=MemorySpace.PSUM) as psm,
        ):
            wcomb = cp.tile([128, HID], BF16)
            nc.sync.dma_start(wcomb[:], wcomb_d[:])
            w2 = cp.tile([HID, HID], BF16)
            nc.sync.dma_start(w2[:], w2_d[:])
            wfc = cp.tile([HID, 1], BF16)
            nc.sync.dma_start(wfc[:], wfc_d[:])
            b1f = cp.tile([HID, 1], F32)
            nc.sync.dma_start(b1f[:], b1f_d[:])
            b2 = cp.tile([HID, 1], F32)
            nc.sync.dma_start(b2[:], b2_d[:])
            outsb = cp.tile([1, OUTPC], F32)

            zt_of, u1_of, x1_of, hs_of, u2_of, x2_of = {}, {}, {}, {}, {}, {}

            def tree16(src_ap, dst_ap, width, tag):
                """Contiguous pairwise s-tree: src [p, 16*width] -> dst [p, width]."""
                a = scp.tile([src_ap.shape[0], 8 * width], BF16, tag=tag + "a")
                nc.vector.tensor_add(a[:], src_ap[:, 0:8 * width],
                                     src_ap[:, 8 * width:16 * width])
                b = scp.tile([src_ap.shape[0], 4 * width], BF16, tag=tag + "b")
                nc.vector.tensor_add(b[:], a[:, 0:4 * width], a[:, 4 * width:8 * width])
                c = scp.tile([src_ap.shape[0], 2 * width], BF16, tag=tag + "c")
                nc.vector.tensor_add(c[:], b[:, 0:2 * width], b[:, 2 * width:4 * width])
                nc.vector.tensor_add(dst_ap, c[:, 0:width], c[:, width:2 * width])

            def stage_a(m):
                zt = ztp.tile([128, MC], BF16, tag="zt")
                zt_of[m] = zt
                nc.sync.dma_start(zt[0:64, :], obs_d[m])
                # graph sums of obs -> Z bottom slot s=0, then log-double
                tree16(zt[0:64, :], zt[64:128, 0:GPM], GPM, "so")
                bot = zt[64:128, :]
                nc.vector.tensor_copy(bot[:, GPM:2 * GPM], bot[:, 0:GPM])
                nc.vector.tensor_copy(bot[:, 2 * GPM:4 * GPM], bot[:, 0:2 * GPM])
                nc.vector.tensor_copy(bot[:, 4 * GPM:8 * GPM], bot[:, 0:4 * GPM])
                nc.vector.tensor_copy(bot[:, 8 * GPM:16 * GPM], bot[:, 0:8 * GPM])

            def stage_b(m, h):
                zt = zt_of[m]
                u1 = pu1.tile([HID, MC // 2], F32, tag="u1")
                u1_of[(m, h)] = u1
                o = h * (MC // 2)
                nc.tensor.matmul(u1[:, 0:512], wcomb[:], zt[:, o:o + 512],
                                 start=True, stop=True)
                nc.tensor.matmul(u1[:, 512:1024], wcomb[:], zt[:, o + 512:o + 1024],
                                 start=True, stop=True)
                if h == 1:
                    zt_of.pop(m)

            def stage_c(m, h):
                u1 = u1_of.pop((m, h))
                if h == 0:
                    x1t = x1p.tile([HID, MC], BF16, tag="x1")
                    x1_of[m] = x1t
                x1 = x1_of[m]
                o = h * (MC // 2)
                nc.scalar.activation(x1[:, o:o + MC // 2], u1[:], AF.Tanh, bias=b1f[:])

            def stage_d(m):
                x1 = x1_of[m]
                sx1 = scp.tile([HID, GPM], BF16, tag="sx1")
                tree16(x1[:], sx1[:], GPM, "s1")
                h2p = psm.tile([HID, GPM], F32, tag="h2")
                nc.tensor.matmul(h2p[:], w2[:], sx1[:], start=True, stop=True)
                h2s = hsp.tile([HID, GPM], BF16, tag="h2s")
                hs_of[m] = h2s
                nc.vector.tensor_scalar_add(h2s[:], h2p[:], b2[:])

            def stage_e(m, h):
                x1 = x1_of[m]
                h2s = hs_of[m]
                if h == 0:
                    u2t = u2p.tile([HID, MC], BF16, tag="u2")
                    u2_of[m] = u2t
                u2 = u2_of[m]
                o = h * (MC // 2)
                hb = h2s[:].rearrange("p (o g) -> p o g", o=1).broadcast_to(
                    [HID, 8, GPM]
                )
                uv = u2[:, o:o + MC // 2].rearrange("p (s g) -> p s g", s=8)
                xv = x1[:, o:o + MC // 2].rearrange("p (s g) -> p s g", s=8)
                nc.gpsimd.tensor_add(uv, xv, hb)
                if h == 1:
                    x1_of.pop(m)
                    hs_of.pop(m)

            def stage_f(m, h):
                u2 = u2_of[m]
                if h == 0:
                    x2t = x2p.tile([HID, MC], BF16, tag="x2")
                    x2_of[m] = x2t
                x2 = x2_of[m]
                o = h * (MC // 2)
                nc.scalar.activation(x2[:, o:o + MC // 2], u2[:, o:o + MC // 2],
                                     AF.Tanh)
                if h == 1:
                    u2_of.pop(m)

            def stage_g(m):
                x2 = x2_of.pop(m)
                sx2 = scp.tile([HID, GPM], BF16, tag="sx2")
                tree16(x2[:], sx2[:], GPM, "s2")
                yp = psm.tile([1, GPM], F32, tag="y")
                nc.tensor.matmul(yp[:], wfc[:], sx2[:], start=True, stop=True)
                nc.scalar.copy(outsb[0:1, m * GPM:(m + 1) * GPM], yp[:])

            # software pipeline over macros (tanh1 runs 2 macros ahead of tanh2)
            stage_a(0); stage_a(1)
            stage_b(0, 0); stage_b(0, 1)
            stage_c(0, 0); stage_c(0, 1)
            stage_a(2)
            stage_b(1, 0); stage_b(1, 1)
            stage_c(1, 0); stage_c(1, 1)
            for m in range(NMC):
                if m + 3 < NMC:
                    stage_a(m + 3)
                if m + 2 < NMC:
                    stage_b(m + 2, 0); stage_b(m + 2, 1)
                    stage_c(m + 2, 0); stage_c(m + 2, 1)
                stage_d(m)
                stage_e(m, 0); stage_e(m, 1)
                stage_f(m, 0); stage_f(m, 1)
                stage_g(m)

            nc.sync.dma_start(out_d[:], outsb[:])

    nc.compile()
    return nc


def _get_nc():
    if "nc" not in _CACHE:
        _CACHE["nc"] = _build_nc()
    return _CACHE["nc"]


def _make_in_maps(cent_obs, w_emb, b_emb, w_gcn, b_gcn, w_fc1):
    w_emb = np.ascontiguousarray(w_emb, np.float32)
    wcomb = np.concatenate(
        [w_emb, (w_emb @ w_gcn[0]) / np.float32(16.0)], axis=0
    ).astype(ml_dtypes.bfloat16)
    w2s = (w_gcn[1] / np.float32(16.0)).astype(ml_dtypes.bfloat16)
    wfc = (w_fc1.reshape(HID, 1) / np.float32(16.0)).astype(ml_dtypes.bfloat16)
    b1f = (b_gcn[0] + b_emb + b_emb @ w_gcn[0]).astype(np.float32).reshape(HID, 1)
    b2 = b_gcn[1].astype(np.float32).reshape(HID, 1)
    shared = {"wcomb": wcomb, "w2s": w2s, "wfc": wfc, "b1f": b1f, "b2": b2}
    obs_all = np.ascontiguousarray(cent_obs, np.float32).astype(ml_dtypes.bfloat16)
    # node n = core*NPC + m*MC + g*16 + s  ->  obs_dev[core, m, :, s*GPM + g]
    obs_all = obs_all.reshape(N_CORES, NMC, GPM, S, OBS).transpose(0, 1, 4, 3, 2)
    obs_all = obs_all.reshape(N_CORES, NMC, OBS, MC)
    in_maps = []
    for ci in range(N_CORES):
        m = dict(shared)
        m["obs"] = np.ascontiguousarray(obs_all[ci])
        in_maps.append(m)
    return in_maps


def kernel(cent_obs, w_emb, b_emb, w_gcn, b_gcn, w_fc1, b_fc1,
           edge_src, edge_dst, _trace=False):
    cent_obs = np.asarray(cent_obs, np.float32)
    nc = _get_nc()
    in_maps = _make_in_maps(
        cent_obs, np.asarray(w_emb, np.float32), np.asarray(b_emb, np.float32),
        np.asarray(w_gcn, np.float32), np.asarray(b_gcn, np.float32),
        np.asarray(w_fc1, np.float32),
    )
    kw = dict(trace=True) if _trace else {}
    res = run_bass_kernel_spmd(nc, in_maps, list(range(N_CORES)), **kw)
    # outsb col (m, g) = graph m*GPM + g  (natural order)
    y = np.concatenate(
        [np.asarray(res.results[i]["out"]).reshape(-1) for i in range(N_CORES)]
    )
    out = (y + np.float32(np.asarray(b_fc1).reshape(()))).astype(np.float32)
    if _trace:
        _CACHE["last_result"] = res
    return out.reshape(BATCH, 1)


# revision 14
# speedup vs baseline: 1.2958x; 1.0650x over previous
"""Trainium2 Bass kernel for GCNCriticNet (gnn_message_passing).

Graphs are 8192 independent complete graphs of 16 nodes (+ self loops), so
every node has degree 16, the symmetric norm is 1/16, and GCN aggregation
collapses to a per-graph mean. Edge lists never reach the device.

Per core (16384 nodes = 1024 graphs), feature-major [128, node-cols], node
columns ordered (s, g) — node-within-graph major — so every reduction /
broadcast is a fully contiguous DVE op:
  u1 = Wcomb^T Z        Z = [obs ; bcast(graph-sum obs)]  (K=128 stacked)
  x1 = tanh(u1 + b1f)                                     ACT
  sx1 = group-sum(x1)   pairwise tree on DVE (bf16 2x)
  h2  = W2s^T sx1       one matmul;  h2s = h2 + b2        DVE
  u2  = x1 + bcast(h2s)                                   GPSIMD
  x2  = tanh(u2)                                          ACT
  y   = wfc^T group-sum(x2)                               DVE tree + matmul
Host: out = y + b_fc1 (mean's /16 folded into weights).

Processed in 8 macro-chunks of 2048 nodes (128 graphs); u1/tanh1/u2/tanh2
split in 1024-col halves to bound PSUM usage and pipeline ACT.
"""

import sys
import numpy as np

try:
    import concourse.bass as bass  # noqa: F401
except ImportError:  # harness runs in a bare dir; repo is on the box
    for p in ("/opt/trn_rl_repo", "/root/.axon_site/_ro/trn_rl_repo"):
        if p not in sys.path:
            sys.path.insert(0, p)
    import concourse.bass as bass  # noqa: F401

import ml_dtypes
import concourse.bacc as bacc
import concourse.mybir as mybir
import concourse.tile as tile
from concourse.bass import MemorySpace
from concourse.bass_utils import run_bass_kernel_spmd

F32 = mybir.dt.float32
BF16 = mybir.dt.bfloat16
AF = mybir.ActivationFunctionType

N_CORES = 8
N_AGENTS = 16
BATCH = 8192
OBS = 64
HID = 128
N = BATCH * N_AGENTS            # 131072 nodes
NPC = N // N_CORES              # 16384 nodes / core
MC = 2048                       # nodes per macro-chunk
NMC = NPC // MC                 # 8
GPM = MC // N_AGENTS            # 128 graphs per macro
OUTPC = NPC // N_AGENTS         # 1024 graphs per core
S = N_AGENTS

_CACHE = {}


def _build_nc():
    nc = bacc.Bacc("TRN2", target_bir_lowering=False, debug=False)

    obs_d = nc.dram_tensor("obs", [NMC, OBS, MC], BF16, kind="ExternalInput")
    wcomb_d = nc.dram_tensor("wcomb", [128, HID], BF16, kind="ExternalInput")
    w2_d = nc.dram_tensor("w2s", [HID, HID], BF16, kind="ExternalInput")
    wfc_d = nc.dram_tensor("wfc", [HID, 1], BF16, kind="ExternalInput")
    b1f_d = nc.dram_tensor("b1f", [HID, 1], F32, kind="ExternalInput")
    b2_d = nc.dram_tensor("b2", [HID, 1], F32, kind="ExternalInput")
    out_d = nc.dram_tensor("out", [1, OUTPC], F32, kind="ExternalOutput")

    with tile.TileContext(nc) as tc:
        with (
            tc.tile_pool(name="const", bufs=1) as cp,
            tc.tile_pool(name="zt", bufs=3) as ztp,
            tc.tile_pool(name="sc", bufs=2) as scp,
            tc.tile_pool(name="x1p", bufs=3) as x1p,
            tc.tile_pool(name="u2p", bufs=2) as u2p,
            tc.tile_pool(name="x2p", bufs=2) as x2p,
            tc.tile_pool(name="hsp", bufs=2) as hsp,
            tc.tile_pool(name="pu1", bufs=2, space=MemorySpace.PSUM) as pu1,
            tc.tile_pool(name="psm", bufs=2, space=MemorySpace.PSUM) as psm,
        ):
            wcomb = cp.tile([128, HID], BF16)
            nc.sync.dma_start(wcomb[:], wcomb_d[:])
            w2 = cp.tile([HID, HID], BF16)
            nc.sync.dma_start(w2[:], w2_d[:])
            wfc = cp.tile([HID, 1], BF16)
            nc.sync.dma_start(wfc[:], wfc_d[:])
            b1f = cp.tile([HID, 1], F32)
            nc.sync.dma_start(b1f[:], b1f_d[:])
            b2 = cp.tile([HID, 1], F32)
            nc.sync.dma_start(b2[:], b2_d[:])
            outsb = cp.tile([1, OUTPC], F32)

            zt_of, u1_of, x1_of, hs_of, u2_of, x2_of = {}, {}, {}, {}, {}, {}

            def tree16(src_ap, dst_ap, width, tag):
                """Contiguous pairwise s-tree: src [p, 16*width] -> dst [p, width]."""
                a = scp.tile([src_ap.shape[0], 8 * width], BF16, tag=tag + "a")
                nc.vector.tensor_add(a[:], src_ap[:, 0:8 * width],
                                     src_ap[:, 8 * width:16 * width])
                b = scp.tile([src_ap.shape[0], 4 * width], BF16, tag=tag + "b")
                nc.vector.tensor_add(b[:], a[:, 0:4 * width], a[:, 4 * width:8 * width])
                c = scp.tile([src_ap.shape[0], 2 * width], BF16, tag=tag + "c")
                nc.vector.tensor_add(c[:], b[:, 0:2 * width], b[:, 2 * width:4 * width])
                nc.vector.tensor_add(dst_ap, c[:, 0:width], c[:, width:2 * width])

            def stage_a(m):
                zt = ztp.tile([128, MC], BF16, tag="zt")
                zt_of[m] = zt
                nc.sync.dma_start(zt[0:64, :], obs_d[m])
                # graph sums of obs -> Z bottom slot s=0; broadcast via DMA
                tree16(zt[0:64, :], zt[64:128, 0:GPM], GPM, "so")
                bot = zt[64:128, :]
                src = bot[:, 0:GPM].rearrange("p (o g) -> p o g", o=1)
                nc.sync.dma_start(
                    bot[:, GPM:16 * GPM].rearrange("p (s g) -> p s g", s=15),
                    src.broadcast_to([64, 15, GPM]),
                )

            def stage_b(m, h):
                zt = zt_of[m]
                u1 = pu1.tile([HID, MC // 2], F32, tag="u1")
                u1_of[(m, h)] = u1
                o = h * (MC // 2)
                nc.tensor.matmul(u1[:, 0:512], wcomb[:], zt[:, o:o + 512],
                                 start=True, stop=True)
                nc.tensor.matmul(u1[:, 512:1024], wcomb[:], zt[:, o + 512:o + 1024],
                                 start=True, stop=True)
                if h == 1:
                    zt_of.pop(m)

            def stage_c(m, h):
                u1 = u1_of.pop((m, h))
                if h == 0:
                    x1t = x1p.tile([HID, MC], BF16, tag="x1")
                    x1_of[m] = x1t
                x1 = x1_of[m]
                o = h * (MC // 2)
                nc.scalar.activation(x1[:, o:o + MC // 2], u1[:], AF.Tanh, bias=b1f[:])

            def stage_d(m):
                x1 = x1_of[m]
                sx1 = scp.tile([HID, GPM], BF16, tag="sx1")
                tree16(x1[:], sx1[:], GPM, "s1")
                h2p = psm.tile([HID, GPM], F32, tag="h2")
                nc.tensor.matmul(h2p[:], w2[:], sx1[:], start=True, stop=True)
                h2s = hsp.tile([HID, GPM], BF16, tag="h2s")
                hs_of[m] = h2s
                nc.scalar.activation(h2s[:], h2p[:], AF.Identity, bias=b2[:])

            def stage_e(m, h):
                x1 = x1_of[m]
                h2s = hs_of[m]
                if h == 0:
                    u2t = u2p.tile([HID, MC], BF16, tag="u2")
                    u2_of[m] = u2t
                u2 = u2_of[m]
                o = h * (MC // 2)
                hb = h2s[:].rearrange("p (o g) -> p o g", o=1).broadcast_to(
                    [HID, 4, GPM]
                )
                for q in range(2):
                    oq = o + q * (MC // 4)
                    uv = u2[:, oq:oq + MC // 4].rearrange("p (s g) -> p s g", s=4)
                    xv = x1[:, oq:oq + MC // 4].rearrange("p (s g) -> p s g", s=4)
                    nc.gpsimd.tensor_add(uv, xv, hb)
                if h == 1:
                    x1_of.pop(m)
                    hs_of.pop(m)

            def stage_f(m, h):
                u2 = u2_of[m]
                if h == 0:
                    x2t = x2p.tile([HID, MC], BF16, tag="x2")
                    x2_of[m] = x2t
                x2 = x2_of[m]
                o = h * (MC // 2)
                nc.scalar.activation(x2[:, o:o + MC // 2], u2[:, o:o + MC // 2],
                                     AF.Tanh)
                if h == 1:
                    u2_of.pop(m)

            def stage_g(m):
                x2 = x2_of.pop(m)
                sx2 = scp.tile([HID, GPM], BF16, tag="sx2")
                tree16(x2[:], sx2[:], GPM, "s2")
                yp = psm.tile([1, GPM], F32, tag="y")
                nc.tensor.matmul(yp[:], wfc[:], sx2[:], start=True, stop=True)
                nc.scalar.copy(outsb[0:1, m * GPM:(m + 1) * GPM], yp[:])

            # software pipeline over macros (tanh1 runs 2 macros ahead of tanh2)
            stage_a(0); stage_a(1)
            stage_b(0, 0); stage_b(0, 1)
            stage_c(0, 0); stage_c(0, 1)
            stage_a(2)
            stage_b(1, 0); stage_b(1, 1)
            stage_c(1, 0); stage_c(1, 1)
            for m in range(NMC):
                if m + 3 < NMC:
                    stage_a(m + 3)
                if m + 2 < NMC:
                    stage_b(m + 2, 0); stage_b(m + 2, 1)
                    stage_c(m + 2, 0); stage_c(m + 2, 1)
                stage_d(m)
                stage_e(m, 0); stage_e(m, 1)
                stage_f(m, 0); stage_f(m, 1)
                stage_g(m)

            nc.sync.dma_start(out_d[:], outsb[:])

    nc.compile()
    return nc


def _get_nc():
    if "nc" not in _CACHE:
        _CACHE["nc"] = _build_nc()
    return _CACHE["nc"]


def _make_in_maps(cent_obs, w_emb, b_emb, w_gcn, b_gcn, w_fc1):
    w_emb = np.ascontiguousarray(w_emb, np.float32)
    wcomb = np.concatenate(
        [w_emb, (w_emb @ w_gcn[0]) / np.float32(16.0)], axis=0
    ).astype(ml_dtypes.bfloat16)
    w2s = (w_gcn[1] / np.float32(16.0)).astype(ml_dtypes.bfloat16)
    wfc = (w_fc1.reshape(HID, 1) / np.float32(16.0)).astype(ml_dtypes.bfloat16)
    b1f = (b_gcn[0] + b_emb + b_emb @ w_gcn[0]).astype(np.float32).reshape(HID, 1)
    b2 = b_gcn[1].astype(np.float32).reshape(HID, 1)
    shared = {"wcomb": wcomb, "w2s": w2s, "wfc": wfc, "b1f": b1f, "b2": b2}
    obs_all = np.ascontiguousarray(cent_obs, np.float32).astype(ml_dtypes.bfloat16)
    # node n = core*NPC + m*MC + g*16 + s  ->  obs_dev[core, m, :, s*GPM + g]
    obs_all = obs_all.reshape(N_CORES, NMC, GPM, S, OBS).transpose(0, 1, 4, 3, 2)
    obs_all = obs_all.reshape(N_CORES, NMC, OBS, MC)
    in_maps = []
    for ci in range(N_CORES):
        m = dict(shared)
        m["obs"] = np.ascontiguousarray(obs_all[ci])
        in_maps.append(m)
    return in_maps


def kernel(cent_obs, w_emb, b_emb, w_gcn, b_gcn, w_fc1, b_fc1,
           edge_src, edge_dst, _trace=False):
    cent_obs = np.asarray(cent_obs, np.float32)
    nc = _get_nc()
    in_maps = _make_in_maps(
        cent_obs, np.asarray(w_emb, np.float32), np.asarray(b_emb, np.float32),
        np.asarray(w_gcn, np.float32), np.asarray(b_gcn, np.float32),
        np.asarray(w_fc1, np.float32),
    )
    kw = dict(trace=True) if _trace else {}
    res = run_bass_kernel_spmd(nc, in_maps, list(range(N_CORES)), **kw)
    # outsb col (m, g) = graph m*GPM + g  (natural order)
    y = np.concatenate(
        [np.asarray(res.results[i]["out"]).reshape(-1) for i in range(N_CORES)]
    )
    out = (y + np.float32(np.asarray(b_fc1).reshape(()))).astype(np.float32)
    if _trace:
        _CACHE["last_result"] = res
    return out.reshape(BATCH, 1)
